# revision 1
# baseline (speedup 1.0000x reference)
"""Trainium2 Bass kernel for nn_MAB_66194035966469 (LIF-gated transformer block).

Strategy: pure data parallelism — batch B=8 maps 1:1 onto the 8 NeuronCores.
Each core computes its full batch element; no collectives.

Per-core math (S=1024 sequence, D=1024 features, H=16 heads, hd=64):
  ATq = lif(Q) = (Q >= 2)                      (binary, exact in any dtype)
  q_pre = ATq @ Wq.T + bq ; q = lif(LN(q_pre)) (LN folded into a per-(row,col)
                                                threshold: q_pre >= mu + c_j*sd)
  same for K -> kbin ; v = lif(K) @ Wv.T + bv
  scores_h = (q_h @ k_h.T) / 8 * mask ; w = softmax ; out_h = w @ v_h
  o = out @ Wo.T + bo ; final = out + mish(o)

Layout: everything feature-major ("T" = transposed, [D, S]) so that every
matmul contraction lands on the partition axis with zero on-device
transposes.  Softmax runs along the free axis via an ones-augmented V
(colsum rides along in the PV matmul) and an indicator-matmul broadcast for
the 1/sum normalization.  mish(o) = o*(u-1)/(u+1), u = (1+e^o)^2, so only
the Exp/Square ACT table is needed (plus Sqrt for the LN sd).

Weight matmuls run in float32r (full-rate fp32 path on TRN2 PE); the binary
QK^T runs in bf16 (exact for 0/1 operands).
"""

import os

import numpy as np
import ml_dtypes

S = 1024          # sequence length (both Sq and Skv)
D = 1024          # feature dim dV
H = 16            # heads
HD = 64           # head dim
NB = D // 128     # 128-partition blocks per feature dim
SH = 512          # s-half (fp32 moving-operand max N)
NCORES = 8
EPS = 1e-5

_CACHE = {}
_LAST = {}


def _patch_tile_drain():
    """This container's walrus rejects instructions carrying more than one
    sem wait.  After Tile finishes scheduling (tail of _drain_and_barrier),
    rewrite every instruction with >1 sync waits so the extra waits ride on
    same-engine NOPs inserted immediately before it."""
    import concourse.mybir as mybir
    from concourse.tile import TileContext

    if getattr(TileContext, "_mab_wait_split", False):
        return
    orig = TileContext._drain_and_barrier

    def split_sync_waits(nc, max_waits=1):
        for func in nc.m.functions:
            for bb in func.blocks:
                insts = bb.instructions
                if not any(
                    i.sync_info and i.sync_info.on_wait
                    and len(i.sync_info.on_wait) > max_waits
                    for i in insts
                ):
                    continue
                out = []
                for inst in insts:
                    si = inst.sync_info
                    if si and si.on_wait and len(si.on_wait) > max_waits:
                        waits = list(si.on_wait)
                        inst.sync_info = mybir.SyncInfo(
                            on_wait=list(waits[:max_waits]),
                            on_update=list(si.on_update),
                        )
                        for w in waits[max_waits:]:
                            ev = mybir.InstEventSemaphore(
                                name=f"I-waitsplit-{nc.next_id()}",
                                ins=[], outs=[],
                            )
                            ev.engine = inst.engine
                            ev.sync_info = mybir.SyncInfo(
                                on_wait=[w], on_update=[]
                            )
                            out.append(ev)
                    out.append(inst)
                bb.instructions[:] = out

    def _patched(self, tick_clock, wait_clock):
        orig(self, tick_clock, wait_clock)
        split_sync_waits(self.nc)

    TileContext._drain_and_barrier = _patched
    TileContext._mab_wait_split = True


def _build():
    import concourse.bass as bass
    import concourse.mybir as mybir
    from concourse.tile import TileContext

    _patch_tile_drain()

    F32 = mybir.dt.float32
    F32R = mybir.dt.float32r
    BF16 = mybir.dt.bfloat16
    AF = mybir.ActivationFunctionType
    ALU = mybir.AluOpType

    nc = bass.Bass()

    # ---- DRAM I/O (per core) ----
    QT = nc.dram_tensor("QT", [D, S], F32, kind="ExternalInput")
    KT = nc.dram_tensor("KT", [D, S], F32, kind="ExternalInput")
    MT = nc.dram_tensor("MT", [S, S], BF16, kind="ExternalInput")   # mask^T [t,s]
    WQT = nc.dram_tensor("WQT", [D, D], F32R, kind="ExternalInput")  # Wq.T [k,n]
    WKT = nc.dram_tensor("WKT", [D, D], F32R, kind="ExternalInput")
    WVT = nc.dram_tensor("WVT", [D, D], F32R, kind="ExternalInput")
    WOT = nc.dram_tensor("WOT", [D, D], F32R, kind="ExternalInput")
    vecs = {}
    for vn in ["BQ", "BK", "BV", "BO", "GQ", "BEQ", "GK", "BEK"]:
        vecs[vn] = nc.dram_tensor(vn, [1, D], F32, kind="ExternalInput")
    INDC = nc.dram_tensor("INDC", [16, D], F32R, kind="ExternalInput")
    ONEC = nc.dram_tensor("ONEC", [128, D], F32R, kind="ExternalInput")
    OUT = nc.dram_tensor("OUT", [D, S], F32, kind="ExternalOutput")  # final^T

    def bc(t):   # bitcast fp32 view -> fp32r for the PE
        return t.bitcast(F32R)

    with TileContext(nc) as tc:
        with (
            nc.allow_low_precision(reason="fp32r tiles feed the PE"),
            tc.tile_pool(name="p0", bufs=1) as p0,           # whole-kernel smalls
            tc.tile_pool(name="pmid", bufs=1) as pmid,       # bin/vaug/mt/unn
        ):
            # ---------- whole-kernel small tensors ----------
            ones128 = p0.tile([128, 1], F32R, tag="ones128")
            nc.sync.dma_start(out=ones128[:], in_=ONEC[:, 0:1])

            # per-partition bias columns, packed: [128, 5, NB]
            # (column group 4 holds the LN epsilon constant)
            colpack = p0.tile([128, 5, NB], F32, tag="colpack")
            bias_cols = {}
            for i, vn in enumerate(["BQ", "BK", "BV", "BO"]):
                nc.sync.dma_start(
                    out=colpack[:, i, :],
                    in_=vecs[vn][0, :].rearrange("(nb p) -> p nb", p=128),
                )
                bias_cols[vn] = colpack[:, i, :]
            # c = (2 - be) / g for q and k (partition-0 rows; engine ops
            # must start at a 32-aligned partition, so no packing here)
            g_t = p0.tile([1, D], F32, tag="g_t")
            be_t = p0.tile([1, D], F32, tag="be_t")
            cvec = {}
            for suf, gn, ben in [("q", "GQ", "BEQ"), ("k", "GK", "BEK")]:
                cv = p0.tile([1, D], F32, tag=f"c_{suf}", name=f"c_{suf}")
                nc.sync.dma_start(out=g_t[:], in_=vecs[gn][:])
                nc.sync.dma_start(out=be_t[:], in_=vecs[ben][:])
                nc.scalar.activation(cv[:], be_t[:], AF.Copy,
                                     bias=0.0, scale=-1.0)
                nc.vector.tensor_scalar_add(cv[:], cv[:], 2.0)
                nc.vector.reciprocal(g_t[:], g_t[:])
                nc.vector.tensor_tensor(cv[:], cv[:], g_t[:], ALU.mult)
                cvec[suf] = cv

            # C2 lhsT for the threshold matmul: [2, NB, 128]
            C2 = {}
            for suf in ["q", "k"]:
                t = p0.tile([2, NB, 128], F32R, tag=f"C2_{suf}")
                nc.sync.dma_start(
                    out=t[0:1, :, :],
                    in_=ONEC[0:1, :].rearrange("o (nb m) -> o nb m", m=128),
                )
                nc.sync.dma_start(
                    out=t[1:2, :, :],
                    in_=cvec[suf][0:1, :].bitcast(F32R).rearrange(
                        "o (nb m) -> o nb m", m=128),
                )
                C2[suf] = t
            # MS rhs [2, S]: row0 = mu, row1 = sd (filled per matrix)
            MS = {}
            for suf in ["q", "k"]:
                MS[suf] = p0.tile([2, S], F32R, tag=f"MS_{suf}", name=f"MS_{suf}")

            # indicator lhsT for softmax-normalization broadcast:
            # IND16[h, kb*128+j] = 1 iff feature kb*128+j belongs to head h
            # (host-provided constant: partition-misaligned memsets are
            # rejected by the BIR verifier)
            IND16 = p0.tile([16, NB, 128], F32R, tag="IND16")
            nc.sync.dma_start(
                out=IND16[:],
                in_=INDC.rearrange("h (nb m) -> h nb m", m=128),
            )

            eps_t = colpack[:, 4, 0:1]
            nc.vector.memset(eps_t, EPS)

            # stats scratch rows (partition 0); MS[suf] is the durable copy
            st_mu = p0.tile([1, S], F32, tag="st_mu")
            st_msq = p0.tile([1, S], F32, tag="st_msq")
            st_sd = p0.tile([1, S], F32, tag="st_sd")

            # softmax colsums + reciprocal [H, S]
            sums = p0.tile([H, S], F32, tag="sums")
            rsum = sums[:].bitcast(F32R)

            # ---------- mid-life big tensors ----------
            qbinT = pmid.tile([128, NB, S], BF16, tag="qbinT")
            kbinT = pmid.tile([128, NB, S], BF16, tag="kbinT")
            v_aug = pmid.tile([128, NB, H * (HD + 1)], F32R, tag="v_aug")
            # ones column of v_aug (for softmax colsums via the PV matmul)
            vview = v_aug[:].rearrange("p tb (h c) -> p tb h c", c=HD + 1)
            for tb in range(NB):
                nc.sync.dma_start(
                    out=vview[:, tb, :, HD:HD + 1],
                    in_=ONEC[:, 0:H].rearrange("p (h o) -> p h o", o=1),
                )

            # =========== Phase 1: FC layers + LN thresholds ===========
            with (
                tc.tile_pool(name="p1", bufs=1) as p1,
                tc.tile_pool(name="pw", bufs=2) as pw,
                tc.tile_pool(name="pwv", bufs=1) as pwv,
                tc.tile_pool(name="pxs", bufs=1) as pxs,
                tc.tile_pool(name="ps1", bufs=4, space="PSUM") as ps1,
                tc.tile_pool(name="ps_st", bufs=1, space="PSUM") as ps_st,
                tc.tile_pool(name="ps_th", bufs=2, space="PSUM") as ps_th,
            ):
                at = {}
                at["q"] = p1.tile([128, NB, S], F32R, tag="at", name="at_q")
                pre = {}

                def load_lif(XT, key):
                    """DMA X^T by 128-row chunk; lif -> at[key] (0/1 fp32)."""
                    for kb in range(NB):
                        xt = pxs.tile([128, S], F32, tag="xs")
                        nc.sync.dma_start(
                            out=xt[:], in_=XT[kb * 128:(kb + 1) * 128, :]
                        )
                        nc.gpsimd.tensor_scalar(
                            at[key][:, kb, :], xt[:], 2.0, None, ALU.is_ge
                        )

                def fc_T(wdram, atk, out_pre, bias_col):
                    """out_pre[n,s] (feature-major) = W @ at + b."""
                    for nb in range(NB):
                        wchunk = pw.tile([128, NB, 128], F32R, tag="wt")
                        nc.sync.dma_start(
                            out=wchunk[:],
                            in_=wdram[:, nb * 128:(nb + 1) * 128].rearrange(
                                "(kb p) m -> p kb m", p=128
                            ),
                        )
                        for sh in range(2):
                            acc = ps1.tile([128, SH], F32, tag="acc")
                            for kb in range(NB):
                                nc.tensor.matmul(
                                    acc[:],
                                    wchunk[:, kb, :],
                                    bc(atk[:, kb, sh * SH:(sh + 1) * SH]),
                                    start=(kb == 0), stop=(kb == NB - 1),
                                )
                            nc.scalar.activation(
                                out_pre[:, nb, sh * SH:(sh + 1) * SH], acc[:],
                                AF.Identity, bias=bias_col[:, nb:nb + 1],
                                scale=1.0,
                            )

                def ln_stats(pre_t, suf):
                    """mu, sd rows of pre_t; then MS rhs."""
                    mu = st_mu[:]
                    msq = st_msq[:]
                    sd = st_sd[:]
                    for sh in range(2):
                        pmu = ps_st.tile([1, SH], F32, tag="pmu")
                        for nb in range(NB):
                            nc.tensor.matmul(
                                pmu[:], bc(ones128[:]),
                                bc(pre_t[:, nb, sh * SH:(sh + 1) * SH]),
                                start=(nb == 0), stop=(nb == NB - 1),
                            )
                        nc.scalar.activation(
                            mu[:, sh * SH:(sh + 1) * SH], pmu[:],
                            AF.Copy, bias=0.0, scale=1.0 / D,
                        )
                    for sh in range(2):
                        psq = ps_st.tile([1, SH], F32, tag="psq")
                        for nb in range(NB):
                            sq = pxs.tile([128, SH], F32R, tag="sq")
                            pslc = pre_t[:, nb,
                                         sh * SH:(sh + 1) * SH].bitcast(F32)
                            nc.gpsimd.tensor_tensor(sq[:], pslc, pslc,
                                                    ALU.mult)
                            nc.tensor.matmul(
                                psq[:], bc(ones128[:]), bc(sq[:]),
                                start=(nb == 0), stop=(nb == NB - 1),
                            )
                        nc.scalar.activation(
                            msq[:, sh * SH:(sh + 1) * SH], psq[:],
                            AF.Copy, bias=0.0, scale=1.0 / D,
                        )
                    mu2 = st_sd[:]   # scratch; overwritten by sqrt below
                    nc.vector.tensor_tensor(mu2, mu, mu, ALU.mult)
                    nc.vector.tensor_tensor(msq, msq, mu2, ALU.subtract)
                    nc.scalar.activation(sd, msq, AF.Sqrt,
                                         bias=eps_t[0:1, :], scale=1.0)
                    # MS row 0 (partition 0): engine copy; row 1: DMA
                    # (engine writes must start at a 32-aligned partition)
                    nc.scalar.activation(MS[suf][0:1, :], mu, AF.Copy)
                    nc.sync.dma_start(out=MS[suf][1:2, :],
                                      in_=sd.bitcast(F32R))

                def lif_norm(pre_t, suf, out_bin):
                    """out_bin = (pre_t >= mu + c*sd), bf16 0/1."""
                    for nb in range(NB):
                        for sh in range(2):
                            th = ps_th.tile([128, SH], F32, tag="th")
                            nc.tensor.matmul(
                                th[:], bc(C2[suf][:, nb, :]),
                                bc(MS[suf][:, sh * SH:(sh + 1) * SH]),
                                start=True, stop=True,
                            )
                            nc.vector.tensor_tensor(
                                out_bin[:, nb, sh * SH:(sh + 1) * SH],
                                pre_t[:, nb,
                                      sh * SH:(sh + 1) * SH].bitcast(F32),
                                th[:], ALU.is_ge,
                            )

                # --- Q side ---
                load_lif(QT, "q")
                pre["q"] = p1.tile([128, NB, S], F32R, tag="pre", name="pre_q")
                fc_T(WQT, at["q"], pre["q"], bias_cols["BQ"])
                ln_stats(pre["q"], "q")
                lif_norm(pre["q"], "q", qbinT)

                # --- K side (reuses the at-slot after fc_q finishes) ---
                at["k"] = p1.tile([128, NB, S], F32R, tag="at", name="at_k")
                load_lif(KT, "k")
                pre["k"] = p1.tile([128, NB, S], F32R, tag="pre", name="pre_k")
                fc_T(WKT, at["k"], pre["k"], bias_cols["BK"])

                # --- fc_v (natural layout v[t, j]; bias bv folded in later
                #     because softmax weights sum to 1) ---
                for jq in range(4):
                    wv = pwv.tile([128, NB, 256], F32R, tag="wtv")
                    nc.sync.dma_start(
                        out=wv[:],
                        in_=WVT[:, jq * 256:(jq + 1) * 256].rearrange(
                            "(kb p) m -> p kb m", p=128
                        ),
                    )
                    for tb in range(NB):
                        accv = ps1.tile([128, 256], F32, tag="acc")
                        for kb in range(NB):
                            nc.tensor.matmul(
                                accv[:],
                                bc(at["k"][:, kb, tb * 128:(tb + 1) * 128]),
                                wv[:, kb, :],
                                start=(kb == 0), stop=(kb == NB - 1),
                            )
                        nc.scalar.activation(
                            vview[:, tb, jq * 4:(jq + 1) * 4, 0:HD],
                            accv[:].rearrange("p (h c) -> p h c", c=HD),
                            AF.Copy,
                        )
                ln_stats(pre["k"], "k")
                lif_norm(pre["k"], "k", kbinT)

            # =========== Phase 2: attention ===========
            with tc.tile_pool(name="punn", bufs=1) as punn:
                unnT = punn.tile([128, NB, S], F32R, tag="unnT")
                with (
                    tc.tile_pool(name="pexp", bufs=3) as pexp,
                    tc.tile_pool(name="pmsc", bufs=3) as pmsc,
                    tc.tile_pool(name="pmt", bufs=3) as pmt,
                    tc.tile_pool(name="ps_sc", bufs=2, space="PSUM") as ps_sc,
                    tc.tile_pool(name="ps_o", bufs=4, space="PSUM") as ps_o,
                ):
                    for h in range(H):
                        po = [ps_o.tile([HD + 1, SH], F32, tag="po",
                                        name=f"po_{h}_{i}")
                              for i in range(2)]
                        pp = (h % 2) * 64
                        hb = h // 2
                        for tb in range(NB):
                            # scoresT[t, s] = sum_j kbinT[j, t] * qbinT[j, s]
                            psc = ps_sc.tile([128, S], F32, tag="psc")
                            for qh in range(2):
                                nc.tensor.matmul(
                                    psc[:, qh * SH:(qh + 1) * SH],
                                    kbinT[pp:pp + 64, hb,
                                          tb * 128:(tb + 1) * 128],
                                    qbinT[pp:pp + 64, hb,
                                          qh * SH:(qh + 1) * SH],
                                    start=True, stop=True,
                                )
                            mt_t = pmt.tile([128, S], BF16, tag="mt_t")
                            nc.sync.dma_start(
                                out=mt_t[:],
                                in_=MT[tb * 128:(tb + 1) * 128, :],
                            )
                            msc = pmsc.tile([128, S], BF16, tag="msc")
                            nc.vector.scalar_tensor_tensor(
                                msc[:], psc[:], 0.125, mt_t[:],
                                ALU.mult, ALU.mult,
                            )
                            expt = pexp.tile([128, S], F32R, tag="expt")
                            nc.scalar.activation(expt[:], msc[:], AF.Exp)
                            for sh in range(2):
                                nc.tensor.matmul(
                                    po[sh][:],
                                    bc(v_aug[:, tb,
                                             h * (HD + 1):(h + 1) * (HD + 1)]),
                                    bc(expt[:, sh * SH:(sh + 1) * SH]),
                                    start=(tb == 0), stop=(tb == NB - 1),
                                )
                        for sh in range(2):
                            nc.scalar.activation(
                                unnT[pp:pp + 64, hb, sh * SH:(sh + 1) * SH],
                                po[sh][0:HD, :], AF.Copy,
                            )
                            stg = pmt.tile([1, SH], F32, tag="stg",
                                           name=f"stg_{h}_{sh}")
                            nc.scalar.activation(
                                stg[:], po[sh][HD:HD + 1, :], AF.Copy,
                            )
                            nc.sync.dma_start(
                                out=sums[h:h + 1, sh * SH:(sh + 1) * SH],
                                in_=stg[:],
                            )

                # softmax normalization: unnT = unnT/sums + bv
                # (multiplicative 1/sum broadcast via a tiny indicator
                # matmul; bv lands exactly because sum(w) == 1)
                nc.vector.reciprocal(rsum, sums[:, :])
                with tc.tile_pool(name="ps_n", bufs=2, space="PSUM") as ps_n:
                    for kb in range(NB):
                        for sh in range(2):
                            pscb = ps_n.tile([128, SH], F32, tag="pscb")
                            nc.tensor.matmul(
                                pscb[:], bc(IND16[:, kb, :]),
                                rsum[:, sh * SH:(sh + 1) * SH],
                                start=True, stop=True,
                            )
                            uslc = unnT[:, kb, sh * SH:(sh + 1) * SH]
                            nc.vector.tensor_tensor(
                                uslc, uslc.bitcast(F32), pscb[:], ALU.mult,
                            )
                            nc.vector.tensor_scalar(
                                uslc, uslc.bitcast(F32),
                                bias_cols["BV"][:, kb:kb + 1], None, ALU.add,
                            )

                # =========== Phase 3: fc_o + mish + residual ===========
                # mish(o) = o * tanh(ln(1 + e^o)).  Exp/Ln share one ACT
                # table set, Tanh needs another; batch tiles in waves so the
                # table switches only a few times.
                with (
                    tc.tile_pool(name="pw3", bufs=3) as pw3,
                    tc.tile_pool(name="po3", bufs=8) as po3,
                    tc.tile_pool(name="psn3", bufs=8) as psn3,
                    tc.tile_pool(name="pfin", bufs=3) as pfin,
                    tc.tile_pool(name="ps3", bufs=4, space="PSUM") as ps3,
                ):
                    for wave in range(2):  # 4 nb x 2 sh tiles per wave
                        o_ts, s_ts, locs = [], [], []
                        for nbq in range(4):
                            nb = wave * 4 + nbq
                            wo = pw3.tile([128, NB, 128], F32R, tag="wto")
                            nc.sync.dma_start(
                                out=wo[:],
                                in_=WOT[:, nb * 128:(nb + 1) * 128].rearrange(
                                    "(kb p) m -> p kb m", p=128
                                ),
                            )
                            for sh in range(2):
                                acc = ps3.tile([128, SH], F32, tag="acc3")
                                for kb in range(NB):
                                    nc.tensor.matmul(
                                        acc[:], wo[:, kb, :],
                                        bc(unnT[:, kb, sh * SH:(sh + 1) * SH]),
                                        start=(kb == 0), stop=(kb == NB - 1),
                                    )
                                o_t = po3.tile([128, SH], F32, tag="o_t")
                                nc.scalar.activation(
                                    o_t[:], acc[:], AF.Identity,
                                    bias=bias_cols["BO"][:, nb:nb + 1],
                                    scale=1.0,
                                )
                                s_t = psn3.tile([128, SH], F32, tag="s_t")
                                # s = ln(1 + e^o)
                                nc.scalar.activation(s_t[:], o_t[:], AF.Exp)
                                nc.scalar.activation(s_t[:], s_t[:], AF.Ln,
                                                     bias=1.0, scale=1.0)
                                o_ts.append(o_t)
                                s_ts.append(s_t)
                                locs.append((nb, sh))
                        for o_t, s_t, (nb, sh) in zip(o_ts, s_ts, locs):
                            nc.scalar.activation(s_t[:], s_t[:], AF.Tanh)
                            m_t = pfin.tile([128, SH], F32, tag="m_t")
                            nc.gpsimd.tensor_tensor(m_t[:], s_t[:], o_t[:],
                                                    ALU.mult)
                            f_t = pfin.tile([128, SH], F32, tag="f_t")
                            nc.gpsimd.tensor_tensor(
                                f_t[:],
                                unnT[:, nb, sh * SH:(sh + 1) * SH].bitcast(F32),
                                m_t[:], ALU.add,
                            )
                            nc.sync.dma_start(
                                out=OUT[nb * 128:(nb + 1) * 128,
                                        sh * SH:(sh + 1) * SH],
                                in_=f_t[:],
                            )
    return nc


def kernel(Q, K, adj_mask, Wq, bq, Wk, bk, Wv, bv, Wo, bo,
           g_q, be_q, g_k, be_k):
    from concourse.bass_utils import run_bass_kernel_spmd

    if "nc" not in _CACHE:
        _CACHE["nc"] = _build()
    nc = _CACHE["nc"]

    f32 = np.float32
    indc = np.zeros((16, D), f32)
    for h in range(H):
        indc[h, h * HD:(h + 1) * HD] = 1.0
    shared = {
        "INDC": indc,
        "ONEC": np.ones((128, D), f32),
        "WQT": np.ascontiguousarray(Wq.T, dtype=f32),
        "WKT": np.ascontiguousarray(Wk.T, dtype=f32),
        "WVT": np.ascontiguousarray(Wv.T, dtype=f32),
        "WOT": np.ascontiguousarray(Wo.T, dtype=f32),
        "BQ": np.ascontiguousarray(bq, dtype=f32).reshape(1, D),
        "BK": np.ascontiguousarray(bk, dtype=f32).reshape(1, D),
        "BV": np.ascontiguousarray(bv, dtype=f32).reshape(1, D),
        "BO": np.ascontiguousarray(bo, dtype=f32).reshape(1, D),
        "GQ": np.ascontiguousarray(g_q, dtype=f32).reshape(1, D),
        "BEQ": np.ascontiguousarray(be_q, dtype=f32).reshape(1, D),
        "GK": np.ascontiguousarray(g_k, dtype=f32).reshape(1, D),
        "BEK": np.ascontiguousarray(be_k, dtype=f32).reshape(1, D),
    }
    in_maps = []
    for b in range(NCORES):
        m = dict(shared)
        m["QT"] = np.ascontiguousarray(np.asarray(Q[b], dtype=f32).T)
        m["KT"] = np.ascontiguousarray(np.asarray(K[b], dtype=f32).T)
        m["MT"] = np.ascontiguousarray(
            np.asarray(adj_mask[b, 0], dtype=f32).T
        ).astype(ml_dtypes.bfloat16)
        in_maps.append(m)

    trace = bool(int(os.environ.get("MAB_TRACE", "0")))
    res = run_bass_kernel_spmd(nc, in_maps, list(range(NCORES)), trace=trace)
    _LAST["res"] = res
    _CACHE["in_maps"] = in_maps
    out = np.stack([res.results[b]["OUT"].T for b in range(NCORES)])
    return np.ascontiguousarray(out).astype(np.float32)


def _make_runner(nc, in_maps, n_cores, loop_iters=1):
    """Replicate bass2jax.run_bass_via_pjrt's sharded execution, but without
    donation and with inputs pre-staged on device, so repeated calls measure
    device execution (plus per-dispatch overhead) only."""
    import jax
    import numpy as np
    import concourse.mybir as mybir
    from jax.sharding import Mesh, NamedSharding, PartitionSpec
    from jax.experimental.shard_map import shard_map
    from concourse.bass2jax import (
        _bass_exec_p, install_neuronx_cc_hook, partition_id_tensor,
    )

    install_neuronx_cc_hook()
    pname = nc.partition_id_tensor.name if nc.partition_id_tensor else None
    in_names, out_names, out_avals, zero_outs = [], [], [], []
    for alloc in nc.m.functions[0].allocations:
        if not isinstance(alloc, mybir.MemoryLocationSet):
            continue
        name = alloc.memorylocations[0].name
        if alloc.kind == "ExternalInput":
            if name != pname:
                in_names.append(name)
        elif alloc.kind == "ExternalOutput":
            out_names.append(name)
            shape = tuple(alloc.tensor_shape)
            dtype = mybir.dt.np(alloc.dtype)
            out_avals.append(jax.core.ShapedArray(shape, dtype))
            zero_outs.append(np.zeros(shape, dtype))
    n_params = len(in_names)
    all_names = in_names + out_names
    if pname is not None:
        all_names = all_names + [pname]

    def _body(*args):
        operands = list(args)
        if pname is not None:
            operands.append(partition_id_tensor())
        outs = _bass_exec_p.bind(
            *operands,
            out_avals=tuple(out_avals),
            in_names=tuple(all_names),
            out_names=tuple(out_names),
            lowering_input_output_aliases=(),
            sim_require_finite=True,
            sim_require_nnan=True,
            nc=nc,
        )
        return tuple(outs)

    devices = jax.devices()[:n_cores]
    mesh = Mesh(np.asarray(devices), ("core",))
    spec = PartitionSpec("core")
    sharded = jax.jit(
        shard_map(_body, mesh=mesh,
                  in_specs=(spec,) * (n_params + len(out_names)),
                  out_specs=(spec,) * len(out_names), check_rep=False),
        keep_unused=True,
    )
    concat = [
        np.concatenate([np.asarray(in_maps[c][nm]) for c in range(n_cores)], axis=0)
        for nm in in_names
    ] + [
        np.zeros((n_cores * z.shape[0], *z.shape[1:]), z.dtype) for z in zero_outs
    ]
    sh = NamedSharding(mesh, spec)
    dev_args = [jax.device_put(a, sh) for a in concat]

    def run(n=1):
        outs = None
        for _ in range(n):
            outs = sharded(*dev_args)
        jax.block_until_ready(outs)
        return outs

    return run


def bench(iters=32, reps=3):
    """Per-execution device time via async-dispatch pipelining: dispatch
    `iters` executions back-to-back and block once; compare with a single
    execution to cancel the dispatch overhead.  Returns (per_exec_s, t1_s)."""
    import time

    assert "nc" in _CACHE and "in_maps" in _CACHE, "run kernel() first"
    run = _make_runner(_CACHE["nc"], _CACHE["in_maps"], NCORES)
    run()

    def timeit(n):
        best = float("inf")
        for _ in range(reps):
            t0 = time.perf_counter()
            run(n)
            best = min(best, time.perf_counter() - t0)
        return best

    t1 = timeit(1)
    tN = timeit(iters)
    per_exec = (tN - t1) / (iters - 1)

    import concourse.bass as bass
    import concourse.mybir as mybir
    if "nc_triv" not in _CACHE:
        nct = bass.Bass()
        xt = nct.dram_tensor("x", [1, 128], mybir.dt.float32,
                             kind="ExternalInput")
        yt = nct.dram_tensor("y", [1, 128], mybir.dt.float32,
                             kind="ExternalOutput")
        from concourse.tile import TileContext
        with TileContext(nct) as tc:
            with tc.tile_pool(name="sb", bufs=1) as sb:
                t = sb.tile([1, 128], mybir.dt.float32, tag="t")
                nct.sync.dma_start(out=t[:], in_=xt[:])
                nct.sync.dma_start(out=yt[:], in_=t[:])
        _CACHE["nc_triv"] = nct
    runt = _make_runner(
        _CACHE["nc_triv"],
        [{"x": np.zeros((1, 128), np.float32)} for _ in range(NCORES)],
        NCORES,
    )
    runt()
    t0 = time.perf_counter()
    runt(1)
    tt1 = time.perf_counter() - t0
    t0 = time.perf_counter()
    runt(iters)
    floor = (time.perf_counter() - t0 - tt1) / (iters - 1)
    return per_exec - max(floor, 0.0), t1

